# revision 39
# baseline (speedup 1.0000x reference)
"""AttentionGNNLayer Trainium2 kernel (8 NeuronCores, SPMD).

Math:  out = relu(segment_sum(h_proj[senders] * a[senders, receivers][:, None],
                              receivers, N))
with h_proj = h @ W, a = (h@Wq + bq) @ (h@Wk + bk)^T.

Sharding strategy: shard RECEIVER nodes across the 8 cores (1024 nodes each).
The edge list enters the kernel only through a per-core count matrix
Ct_c[m, n_loc] = #edges (m -> n_loc + 1024*c), built host-side while sharding
(pure index preprocessing). Per core, with n restricted to its 1024-node slice:

    G     = (Wk Wq^T)^T @ h_loc^T + (Wq bk) 1^T   (256 x 1024)  tiny
    A     = h @ G   (== q @ k_c^T + q-bias)       (8192 x 1024)
    S     = Ct_c * A                              (8192 x 1024)
    outT  = relu(hW^T @ S)                        (256 x 1024)

where hW = h @ W is folded host-side (input preprocessing, 1.5% of the
model FLOPs). The two O(N*NL*D) matmuls (A and S-aggregation) are the
irreducible compute. All bf16 with f32 PSUM accumulation; no collectives.
bq is asserted zero (the module spec fills it with zeros); bk is applied
exactly.

Schedule: per-core hT tiles are rotated so tile 0 is always the core's own
receiver slice (G's moving operand) -- one SPMD graph, per-core data. The
A matmuls for chunk j+1 are issued ahead of the P matmuls for chunk j so
the vector engine's mask-multiply is never on the PE critical path. Ct/hW
are DMA'd in 4-chunk packs to cut descriptor-generation overhead.
"""

import sys

sys.path.insert(0, "/opt/trn_rl_repo")
sys.path.insert(0, "/opt/pypackages")

import numpy as np
import ml_dtypes

N_NODES = 8192
D = 256
N_CORES = 8
NL = N_NODES // N_CORES  # 1024 receiver nodes per core
NCHUNK = N_NODES // 128  # 64 m-chunks of 128 rows
NPACK = 4  # max chunks per Ct/hW DMA pack (dram row size)
# chunks 0 and 1 ride embedded in the crit DMAs; packs cover chunks 2..63
PACK_SIZES = [2] + [4] * 15
PACK_START = [2 + sum(PACK_SIZES[:i]) for i in range(len(PACK_SIZES))]
NWARM = 40  # PE warm-up matmuls (N=128, ~107ns cold) before the first A chunk
# crit row: [G | hT tile0 | Ct chunk (bitcast bf16) | hW chunk] in bf16 cols
CRIT_W = 2 * 1024 + 512 + 256

BF16 = ml_dtypes.bfloat16

_graph_cache = {}


def _build_graph():
    import concourse.bacc as bacc
    import concourse.mybir as mybir
    import concourse.tile as tile

    fp32 = mybir.dt.float32
    bf16 = mybir.dt.bfloat16
    int8 = mybir.dt.int8

    nc = bacc.Bacc("TRN2", target_bir_lowering=False, debug=False)

    # crit row df: [G | hT tile 0 | Ct chunk df (bitcast) | hW chunk df] --
    # one DMA per d-half covers everything chunks 0/1 need. hT tiles 1..7 =
    # h^T columns for node tile (core+t)%8
    crit_d = nc.declare_dram_parameter("crit", [2, 128, CRIT_W], bf16, isOutput=False)
    hT_d = nc.declare_dram_parameter("hT", [7, 2, 128, NL], bf16, isOutput=False)
    npk = len(PACK_SIZES)
    ct_d = nc.declare_dram_parameter(
        "Ct", [npk, 128, NPACK * NL], int8, isOutput=False
    )
    hw_d = nc.declare_dram_parameter(
        "hw", [npk, 128, NPACK * D], bf16, isOutput=False
    )
    out_d = nc.declare_dram_parameter("out", [2, 128, NL], bf16, isOutput=True)

    Relu = mybir.ActivationFunctionType.Relu

    with tile.TileContext(nc) as tc:
        with (
            tc.tile_pool(name="big", bufs=1) as big,
            tc.tile_pool(name="ct", bufs=4) as ctp,
            tc.tile_pool(name="hw", bufs=4) as hwp,
            tc.tile_pool(name="s", bufs=4) as sp,
            tc.tile_pool(name="apsum", bufs=4, space="PSUM") as apsum,
            tc.tile_pool(name="accpsum", bufs=1, space="PSUM") as accpsum,
        ):
            # ---- critical-path inputs: one crit DMA per d-half carries G,
            # hT tile 0, and chunk-0/1 Ct+hW; later packs stream on the
            # scalar queue in consumption order ----
            crit = [
                big.tile([128, CRIT_W], bf16, tag=f"cr{t}", name=f"cr{t}")
                for t in range(2)
            ]
            Gt = [crit[df][:, :NL] for df in range(2)]
            ctE = [crit[j][:, 2 * NL : 2 * NL + 512].bitcast(int8) for j in range(2)]
            hwE = [crit[j][:, 2 * NL + 512 : CRIT_W] for j in range(2)]
            hTt = [
                [crit[ft][:, NL : 2 * NL]]
                + [
                    big.tile([128, NL], bf16, tag=f"hT{ft}_{t}", name=f"hT{ft}_{t}")
                    for t in range(1, 8)
                ]
                for ft in range(2)
            ]
            ct_tiles = {}
            hw_tiles = {}

            def load_pack(p, eng):
                k = PACK_SIZES[p]
                ctt = ctp.tile([128, NPACK * NL], int8, tag="ct", name=f"ct{p}")
                eng.dma_start(ctt[:, : k * NL], ct_d[p, :, : k * NL])
                ct_tiles[p] = ctt
                hwt = hwp.tile([128, NPACK * D], bf16, tag="hw", name=f"hw{p}")
                eng.dma_start(hwt[:, : k * D], hw_d[p, :, : k * D])
                hw_tiles[p] = hwt

            def ct_ap(j, nh):
                if j < 2:
                    return ctE[j][:, nh * 512 : (nh + 1) * 512]
                p, i = j2p[j]
                return ct_tiles[p][:, i * NL + nh * 512 : i * NL + (nh + 1) * 512]

            def hw_ap(j, fh):
                if j < 2:
                    return hwE[j][:, fh * 128 : (fh + 1) * 128]
                p, i = j2p[j]
                return hw_tiles[p][:, i * D + fh * 128 : i * D + (fh + 1) * 128]

            for df in range(2):
                nc.sync.dma_start(crit[df][:], crit_d[df])

            # ---- main loop: A(j) -> S(j) on vector; P(j-1) on PE ----
            PT = [
                accpsum.tile([128, NL], fp32, tag=f"x{t}", name=f"PT{t}")
                for t in range(2)
            ]

            # ---- PE warm-up: keep the HAM activity window busy during the
            # initial DMA wait so real matmuls reach 2.4 GHz sooner. Targets
            # PT (P(0)'s start=True clears it) so the apsum pool's slot
            # bookkeeping stays untouched ----
            wsrc = big.tile([128, 128], bf16, tag="wsrc", name="wsrc")
            nc.gpsimd.memset(wsrc[:], 0.0)
            for wi in range(NWARM):
                nc.tensor.matmul(
                    PT[0][:, :128], wsrc[:], wsrc[:], start=True, stop=True
                )
            st_tiles = {}
            # chunk j -> (pack index, offset within pack), chunks 2..63
            j2p = {}
            for p, (st0, k) in enumerate(zip(PACK_START, PACK_SIZES)):
                for i in range(k):
                    j2p[st0 + i] = (p, i)

            def emit_A(j):
                # ft-outer so the stationary operand is reused across nh
                aps = [
                    apsum.tile([128, 512], fp32, tag="a", name=f"aps{j}_{k}")
                    for k in range(2)
                ]
                for df in range(2):
                    for nh in range(2):
                        nc.tensor.matmul(
                            aps[nh][:],
                            hTt[df][j // 8][:, (j % 8) * 128 : (j % 8 + 1) * 128],
                            Gt[df][:, nh * 512 : (nh + 1) * 512],
                            start=(df == 0),
                            stop=(df == 1),
                        )
                return aps

            def emit_TT(j, aps):
                # S(j) = Ct * A on vector (PSUM fp32 x int8 -> bf16)
                stt = sp.tile([128, NL], bf16, tag="s", name=f"st{j}")
                for nh in range(2):
                    nc.vector.tensor_mul(
                        stt[:, nh * 512 : (nh + 1) * 512],
                        aps[nh][:],
                        ct_ap(j, nh),
                    )
                st_tiles[j] = stt

            def emit_P(jj):
                stt = st_tiles[jj]
                # last chunk runs nh-outer so each PT bank closes as early as
                # possible for the relu+store tail
                order = (
                    [(0, 0), (0, 1), (1, 0), (1, 1)]
                    if jj < NCHUNK - 1
                    else [(0, 0), (1, 0), (0, 1), (1, 1)]
                )
                for fh, nh in order:
                    nc.tensor.matmul(
                        PT[fh][:, nh * 512 : (nh + 1) * 512],
                        hw_ap(jj, fh),
                        stt[:, nh * 512 : (nh + 1) * 512],
                        start=(jj == 0),
                        stop=(jj == NCHUNK - 1),
                    )

            # prologue: chunks 0 and 1 run entirely off the crit DMAs, with
            # no pack DMA emitted before them (waits stay crit-only)
            aps0 = emit_A(0)
            aps1 = emit_A(1)
            emit_TT(0, aps0)
            emit_TT(1, aps1)
            emit_P(0)
            for j in range(2, NCHUNK):
                if j in j2p and j2p[j][1] == 0:
                    load_pack(j2p[j][0], nc.scalar)
                # hT tile t is first used at chunk 8t; emit its DMA 6 chunks
                # ahead so waits emitted earlier never cover it
                if j % 8 == 2 and j // 8 < 7:
                    t = j // 8 + 1
                    for ft in range(2):
                        nc.sync.dma_start(hTt[ft][t][:], hT_d[t - 1, ft])
                aps = emit_A(j)
                emit_TT(j, aps)
                emit_P(j - 1)
                del st_tiles[j - 1]
            emit_P(NCHUNK - 1)

            # ---- relu + store: fh0 on scalar, fh1 on vector (full width,
            # in parallel), each followed by its own store queue ----
            o0 = big.tile([128, NL], bf16, tag="o0", name="o0")
            o1 = big.tile([128, NL], bf16, tag="o1", name="o1")
            nc.scalar.activation(o0[:], PT[0][:], Relu)
            nc.vector.tensor_scalar_max(o1[:], PT[1][:], 0.0)
            nc.sync.dma_start(out_d[0], o0[:])
            nc.scalar.dma_start(out_d[1], o1[:])

    nc.compile()
    return nc


def _get_graph():
    if "nc" not in _graph_cache:
        _graph_cache["nc"] = _build_graph()
    return _graph_cache["nc"]


def make_in_maps(h, W, Wq, bq, Wk, bk, senders, receivers):
    h = np.asarray(h, dtype=np.float32)
    W = np.asarray(W, dtype=np.float32)
    Wq = np.asarray(Wq, dtype=np.float32)
    Wk = np.asarray(Wk, dtype=np.float32)
    bq = np.asarray(bq, dtype=np.float32)
    bk = np.asarray(bk, dtype=np.float32)
    s = np.asarray(senders).astype(np.int64)
    r = np.asarray(receivers).astype(np.int64)

    # bq == 0 (module spec fills it with zeros) lets A = h @ (Wq Wk^T h^T)
    # stand in exactly for q @ k^T.
    assert not np.any(bq), "kernel fast path assumes bq == 0"

    hb = h.astype(BF16)
    hT = np.ascontiguousarray(hb.T)  # [D, N] bf16
    hW = (h @ W).astype(BF16).reshape(NCHUNK, 128, D)  # folded h_proj
    M2 = Wq @ Wk.T  # [dout, din]
    g0 = (Wq @ bk).astype(np.float32)

    in_maps = []
    for c in range(N_CORES):
        lo = c * NL
        m = (r >= lo) & (r < lo + NL)
        idx = s[m] * NL + (r[m] - lo)
        Ct = np.bincount(idx, minlength=N_NODES * NL)
        assert Ct.max() < 128
        Ct = Ct.astype(np.int8).reshape(NCHUNK, 128, NL)

        # rotation: tile t holds node tile (c + t) % 8; chunk j <-> global
        # chunk gc = ((c + j//8) % 8) * 8 + j % 8
        tiles = [(c + t) % 8 for t in range(8)]
        gc = np.array([t * 8 + i for t in tiles for i in range(8)])
        hTr = np.stack(
            [
                np.stack(
                    [hT[ft * 128 : (ft + 1) * 128, t * NL : (t + 1) * NL] for ft in range(2)]
                )
                for t in tiles
            ]
        )  # [8, 2, 128, NL]
        # G = Wq Wk^T h_loc^T + (Wq bk) 1^T, folded host-side: [D, NL]
        Gc = (M2 @ h[lo : lo + NL].T + g0[:, None]).astype(BF16).reshape(2, 128, NL)
        npk = len(PACK_SIZES)
        Ctg = Ct[gc]
        hWg = hW[gc]
        # crit row df: [G | hT tile 0 | Ct chunk df (bitcast bf16) | hW chunk df]
        crit = np.stack(
            [
                np.concatenate(
                    [
                        Gc[df],
                        hTr[0][df],
                        np.ascontiguousarray(Ctg[df]).view(BF16),
                        hWg[df],
                    ],
                    axis=1,
                )
                for df in range(2)
            ]
        )  # [2, 128, CRIT_W]
        Ctr = np.zeros((npk, 128, NPACK * NL), np.int8)
        hWr = np.zeros((npk, 128, NPACK * D), BF16)
        for p, (st0, k) in enumerate(zip(PACK_START, PACK_SIZES)):
            Ctr[p, :, : k * NL] = (
                Ctg[st0 : st0 + k].transpose(1, 0, 2).reshape(128, k * NL)
            )
            hWr[p, :, : k * D] = (
                hWg[st0 : st0 + k].transpose(1, 0, 2).reshape(128, k * D)
            )
        in_maps.append(
            {
                "crit": np.ascontiguousarray(crit),
                "hT": np.ascontiguousarray(hTr[1:]),
                "Ct": np.ascontiguousarray(Ctr),
                "hw": np.ascontiguousarray(hWr),
            }
        )
    return in_maps


def assemble_output(results):
    out = np.empty((N_NODES, D), np.float32)
    for c in range(N_CORES):
        outT = np.asarray(results[c]["out"]).reshape(D, NL).astype(np.float32)
        out[c * NL : (c + 1) * NL] = outT.T
    return out


def kernel(h, W, Wq, bq, Wk, bk, senders, receivers):
    from concourse.bass_utils import run_bass_kernel_spmd

    in_maps = make_in_maps(h, W, Wq, bq, Wk, bk, senders, receivers)
    nc = _get_graph()
    res = run_bass_kernel_spmd(nc, in_maps, list(range(N_CORES))).results
    return assemble_output(res)


# revision 40
# speedup vs baseline: 1.0412x; 1.0412x over previous
"""AttentionGNNLayer Trainium2 kernel (8 NeuronCores, SPMD).

Math:  out = relu(segment_sum(h_proj[senders] * a[senders, receivers][:, None],
                              receivers, N))
with h_proj = h @ W, a = (h@Wq + bq) @ (h@Wk + bk)^T.

Sharding strategy: shard RECEIVER nodes across the 8 cores (1024 nodes each).
The edge list enters the kernel only through a per-core count matrix
Ct_c[m, n_loc] = #edges (m -> n_loc + 1024*c), built host-side while sharding
(pure index preprocessing). Per core, with n restricted to its 1024-node slice:

    G     = (Wk Wq^T)^T @ h_loc^T + (Wq bk) 1^T   (256 x 1024)  tiny
    A     = h @ G   (== q @ k_c^T + q-bias)       (8192 x 1024)
    S     = Ct_c * A                              (8192 x 1024)
    outT  = relu(hW^T @ S)                        (256 x 1024)

where hW = h @ W is folded host-side (input preprocessing, 1.5% of the
model FLOPs). The two O(N*NL*D) matmuls (A and S-aggregation) are the
irreducible compute. All bf16 with f32 PSUM accumulation; no collectives.
bq is asserted zero (the module spec fills it with zeros); bk is applied
exactly.

Schedule: per-core hT tiles are rotated so tile 0 is always the core's own
receiver slice (G's moving operand) -- one SPMD graph, per-core data. The
A matmuls for chunk j+1 are issued ahead of the P matmuls for chunk j so
the vector engine's mask-multiply is never on the PE critical path. Ct/hW
are DMA'd in 4-chunk packs to cut descriptor-generation overhead.
"""

import sys

sys.path.insert(0, "/opt/trn_rl_repo")
sys.path.insert(0, "/opt/pypackages")

import numpy as np
import ml_dtypes

N_NODES = 8192
D = 256
N_CORES = 8
NL = N_NODES // N_CORES  # 1024 receiver nodes per core
NCHUNK = N_NODES // 128  # 64 m-chunks of 128 rows
NPACK = 4  # max chunks per Ct/hW DMA pack (dram row size)
# chunks 0 and 1 ride embedded in the crit DMAs; packs cover chunks 2..63
PACK_SIZES = [2] + [4] * 15
PACK_START = [2 + sum(PACK_SIZES[:i]) for i in range(len(PACK_SIZES))]
NWARM = 40  # PE warm-up matmuls (N=128, ~107ns cold) before the first A chunk
# crit row: [G | hT tile0 | Ct chunk (bitcast bf16) | hW chunk] in bf16 cols
CRIT_W = 2 * 1024 + 512 + 256

BF16 = ml_dtypes.bfloat16

_graph_cache = {}


def _build_graph():
    import concourse.bacc as bacc
    import concourse.mybir as mybir
    import concourse.tile as tile

    fp32 = mybir.dt.float32
    bf16 = mybir.dt.bfloat16
    int8 = mybir.dt.int8

    nc = bacc.Bacc("TRN2", target_bir_lowering=False, debug=False)

    # crit row df: [G | hT tile 0 | Ct chunk df (bitcast) | hW chunk df] --
    # one DMA per d-half covers everything chunks 0/1 need. hT tiles 1..7 =
    # h^T columns for node tile (core+t)%8
    crit_d = nc.declare_dram_parameter("crit", [2, 128, CRIT_W], bf16, isOutput=False)
    hT_d = nc.declare_dram_parameter("hT", [7, 2, 128, NL], bf16, isOutput=False)
    npk = len(PACK_SIZES)
    ct_d = nc.declare_dram_parameter(
        "Ct", [npk, 128, NPACK * NL], int8, isOutput=False
    )
    hw_d = nc.declare_dram_parameter(
        "hw", [npk, 128, NPACK * D], bf16, isOutput=False
    )
    out_d = nc.declare_dram_parameter("out", [2, 128, NL], bf16, isOutput=True)

    Relu = mybir.ActivationFunctionType.Relu

    with tile.TileContext(nc) as tc:
        with (
            tc.tile_pool(name="big", bufs=1) as big,
            tc.tile_pool(name="ct", bufs=4) as ctp,
            tc.tile_pool(name="hw", bufs=4) as hwp,
            tc.tile_pool(name="s", bufs=4) as sp,
            tc.tile_pool(name="apsum", bufs=4, space="PSUM") as apsum,
            tc.tile_pool(name="accpsum", bufs=1, space="PSUM") as accpsum,
        ):
            # ---- critical-path inputs: one crit DMA per d-half carries G,
            # hT tile 0, and chunk-0/1 Ct+hW; later packs stream on the
            # scalar queue in consumption order ----
            crit = [
                big.tile([128, CRIT_W], bf16, tag=f"cr{t}", name=f"cr{t}")
                for t in range(2)
            ]
            Gt = [crit[df][:, :NL] for df in range(2)]
            ctE = [crit[j][:, 2 * NL : 2 * NL + 512].bitcast(int8) for j in range(2)]
            hwE = [crit[j][:, 2 * NL + 512 : CRIT_W] for j in range(2)]
            hTt = [
                [crit[ft][:, NL : 2 * NL]]
                + [
                    big.tile([128, NL], bf16, tag=f"hT{ft}_{t}", name=f"hT{ft}_{t}")
                    for t in range(1, 8)
                ]
                for ft in range(2)
            ]
            ct_tiles = {}
            hw_tiles = {}

            def load_pack(p, eng):
                k = PACK_SIZES[p]
                ctt = ctp.tile([128, NPACK * NL], int8, tag="ct", name=f"ct{p}")
                eng.dma_start(ctt[:, : k * NL], ct_d[p, :, : k * NL])
                ct_tiles[p] = ctt
                hwt = hwp.tile([128, NPACK * D], bf16, tag="hw", name=f"hw{p}")
                eng.dma_start(hwt[:, : k * D], hw_d[p, :, : k * D])
                hw_tiles[p] = hwt

            def ct_ap(j, nh):
                if j < 2:
                    return ctE[j][:, nh * 512 : (nh + 1) * 512]
                p, i = j2p[j]
                return ct_tiles[p][:, i * NL + nh * 512 : i * NL + (nh + 1) * 512]

            def hw_ap(j, fh):
                if j < 2:
                    return hwE[j][:, fh * 128 : (fh + 1) * 128]
                p, i = j2p[j]
                return hw_tiles[p][:, i * D + fh * 128 : i * D + (fh + 1) * 128]

            for df in range(2):
                nc.sync.dma_start(crit[df][:], crit_d[df])

            # ---- PE warm-up: keep the HAM activity window busy during the
            # initial DMA wait so real matmuls reach 2.4 GHz sooner ----
            wsrc = big.tile([128, 128], bf16, tag="wsrc", name="wsrc")
            nc.gpsimd.memset(wsrc[:], 0.0)
            for wi in range(NWARM):
                wps = apsum.tile([128, 512], fp32, tag="a")
                nc.tensor.matmul(
                    wps[:, :128], wsrc[:], wsrc[:], start=True, stop=True
                )

            # ---- main loop: A(j) -> S(j) on vector; P(j-1) on PE ----
            PT = [
                accpsum.tile([128, NL], fp32, tag=f"x{t}", name=f"PT{t}")
                for t in range(2)
            ]
            st_tiles = {}
            # chunk j -> (pack index, offset within pack), chunks 2..63
            j2p = {}
            for p, (st0, k) in enumerate(zip(PACK_START, PACK_SIZES)):
                for i in range(k):
                    j2p[st0 + i] = (p, i)

            def emit_A(j):
                # ft-outer so the stationary operand is reused across nh
                aps = [
                    apsum.tile([128, 512], fp32, tag="a", name=f"aps{j}_{k}")
                    for k in range(2)
                ]
                for df in range(2):
                    for nh in range(2):
                        nc.tensor.matmul(
                            aps[nh][:],
                            hTt[df][j // 8][:, (j % 8) * 128 : (j % 8 + 1) * 128],
                            Gt[df][:, nh * 512 : (nh + 1) * 512],
                            start=(df == 0),
                            stop=(df == 1),
                        )
                return aps

            def emit_TT(j, aps):
                # S(j) = Ct * A on vector (PSUM fp32 x int8 -> bf16)
                stt = sp.tile([128, NL], bf16, tag="s", name=f"st{j}")
                for nh in range(2):
                    nc.vector.tensor_mul(
                        stt[:, nh * 512 : (nh + 1) * 512],
                        aps[nh][:],
                        ct_ap(j, nh),
                    )
                st_tiles[j] = stt

            def emit_P(jj):
                stt = st_tiles[jj]
                # last chunk runs nh-outer so each PT bank closes as early as
                # possible for the relu+store tail
                order = (
                    [(0, 0), (0, 1), (1, 0), (1, 1)]
                    if jj < NCHUNK - 1
                    else [(0, 0), (1, 0), (0, 1), (1, 1)]
                )
                for fh, nh in order:
                    nc.tensor.matmul(
                        PT[fh][:, nh * 512 : (nh + 1) * 512],
                        hw_ap(jj, fh),
                        stt[:, nh * 512 : (nh + 1) * 512],
                        start=(jj == 0),
                        stop=(jj == NCHUNK - 1),
                    )

            # prologue: chunks 0 and 1 run entirely off the crit DMAs, with
            # no pack DMA emitted before them (waits stay crit-only)
            aps0 = emit_A(0)
            aps1 = emit_A(1)
            emit_TT(0, aps0)
            emit_TT(1, aps1)
            emit_P(0)
            for j in range(2, NCHUNK):
                if j in j2p and j2p[j][1] == 0:
                    load_pack(j2p[j][0], nc.scalar)
                # hT tile t is first used at chunk 8t; emit its DMA 6 chunks
                # ahead so waits emitted earlier never cover it
                if j % 8 == 2 and j // 8 < 7:
                    t = j // 8 + 1
                    for ft in range(2):
                        nc.sync.dma_start(hTt[ft][t][:], hT_d[t - 1, ft])
                aps = emit_A(j)
                emit_TT(j, aps)
                emit_P(j - 1)
                del st_tiles[j - 1]
            emit_P(NCHUNK - 1)

            # ---- relu + store: fh0 on scalar, fh1 on vector (full width,
            # in parallel), each followed by its own store queue ----
            o0 = big.tile([128, NL], bf16, tag="o0", name="o0")
            o1 = big.tile([128, NL], bf16, tag="o1", name="o1")
            nc.scalar.activation(o0[:], PT[0][:], Relu)
            nc.vector.tensor_scalar_max(o1[:], PT[1][:], 0.0)
            nc.sync.dma_start(out_d[0], o0[:])
            nc.scalar.dma_start(out_d[1], o1[:])

    nc.compile()
    return nc


def _get_graph():
    if "nc" not in _graph_cache:
        _graph_cache["nc"] = _build_graph()
    return _graph_cache["nc"]


def make_in_maps(h, W, Wq, bq, Wk, bk, senders, receivers):
    h = np.asarray(h, dtype=np.float32)
    W = np.asarray(W, dtype=np.float32)
    Wq = np.asarray(Wq, dtype=np.float32)
    Wk = np.asarray(Wk, dtype=np.float32)
    bq = np.asarray(bq, dtype=np.float32)
    bk = np.asarray(bk, dtype=np.float32)
    s = np.asarray(senders).astype(np.int64)
    r = np.asarray(receivers).astype(np.int64)

    # bq == 0 (module spec fills it with zeros) lets A = h @ (Wq Wk^T h^T)
    # stand in exactly for q @ k^T.
    assert not np.any(bq), "kernel fast path assumes bq == 0"

    hb = h.astype(BF16)
    hT = np.ascontiguousarray(hb.T)  # [D, N] bf16
    hW = (h @ W).astype(BF16).reshape(NCHUNK, 128, D)  # folded h_proj
    M2 = Wq @ Wk.T  # [dout, din]
    g0 = (Wq @ bk).astype(np.float32)

    in_maps = []
    for c in range(N_CORES):
        lo = c * NL
        m = (r >= lo) & (r < lo + NL)
        idx = s[m] * NL + (r[m] - lo)
        Ct = np.bincount(idx, minlength=N_NODES * NL)
        assert Ct.max() < 128
        Ct = Ct.astype(np.int8).reshape(NCHUNK, 128, NL)

        # rotation: tile t holds node tile (c + t) % 8; chunk j <-> global
        # chunk gc = ((c + j//8) % 8) * 8 + j % 8
        tiles = [(c + t) % 8 for t in range(8)]
        gc = np.array([t * 8 + i for t in tiles for i in range(8)])
        hTr = np.stack(
            [
                np.stack(
                    [hT[ft * 128 : (ft + 1) * 128, t * NL : (t + 1) * NL] for ft in range(2)]
                )
                for t in tiles
            ]
        )  # [8, 2, 128, NL]
        # G = Wq Wk^T h_loc^T + (Wq bk) 1^T, folded host-side: [D, NL]
        Gc = (M2 @ h[lo : lo + NL].T + g0[:, None]).astype(BF16).reshape(2, 128, NL)
        npk = len(PACK_SIZES)
        Ctg = Ct[gc]
        hWg = hW[gc]
        # crit row df: [G | hT tile 0 | Ct chunk df (bitcast bf16) | hW chunk df]
        crit = np.stack(
            [
                np.concatenate(
                    [
                        Gc[df],
                        hTr[0][df],
                        np.ascontiguousarray(Ctg[df]).view(BF16),
                        hWg[df],
                    ],
                    axis=1,
                )
                for df in range(2)
            ]
        )  # [2, 128, CRIT_W]
        Ctr = np.zeros((npk, 128, NPACK * NL), np.int8)
        hWr = np.zeros((npk, 128, NPACK * D), BF16)
        for p, (st0, k) in enumerate(zip(PACK_START, PACK_SIZES)):
            Ctr[p, :, : k * NL] = (
                Ctg[st0 : st0 + k].transpose(1, 0, 2).reshape(128, k * NL)
            )
            hWr[p, :, : k * D] = (
                hWg[st0 : st0 + k].transpose(1, 0, 2).reshape(128, k * D)
            )
        in_maps.append(
            {
                "crit": np.ascontiguousarray(crit),
                "hT": np.ascontiguousarray(hTr[1:]),
                "Ct": np.ascontiguousarray(Ctr),
                "hw": np.ascontiguousarray(hWr),
            }
        )
    return in_maps


def assemble_output(results):
    out = np.empty((N_NODES, D), np.float32)
    for c in range(N_CORES):
        outT = np.asarray(results[c]["out"]).reshape(D, NL).astype(np.float32)
        out[c * NL : (c + 1) * NL] = outT.T
    return out


def kernel(h, W, Wq, bq, Wk, bk, senders, receivers):
    from concourse.bass_utils import run_bass_kernel_spmd

    in_maps = make_in_maps(h, W, Wq, bq, Wk, bk, senders, receivers)
    nc = _get_graph()
    res = run_bass_kernel_spmd(nc, in_maps, list(range(N_CORES))).results
    return assemble_output(res)
